# revision 14
# baseline (speedup 1.0000x reference)
"""Trainium2 Bass kernel for nn_BodyAvgDiseaseFeatureAttn2.

Computation (reference):
    attn  = softmax over channels of [heart(27); lung(28); lung(28)] -> [83, 16]
    Weff[o,c,h,w] = attn[o,c] * Wfc[o,c,h,w]
    out[b,o] = mean_s( sum_{c,h,w} x[b,s,c,h,w] * Weff[o,c,h,w] ) + bias[o]

Kernel strategy (pure data parallel, 8 cores, batch-sharded), v2:
  - x per core is 17.7 MB; HBM stream at ~428 GB/s is the roofline
    (~41.5 us). All 4 batch tiles (138 KB/partition) fit in SBUF, so ALL
    x DMAs are issued up front (20 chunk DMAs, no buffer reuse -> no
    stalls), and compute overlaps the stream with large headroom.
  - slice-sum tree: fp32 pair-adds split DVE (chunks 0,1 + combines) /
    GPSIMD (chunk 2 + tail combines), fused via 3D APs (2 pairs/op).
  - final 576-dot on PE in BF16 (tree output downcast in its last add;
    Weff precomputed in bf16): transposes+matmuls ~2x faster, fp32 PSUM
    accumulation keeps error ~1e-3 << 2e-2 budget.
  - psum->sbuf copies and the bias add run on the Scalar engine.
  - output store split into 4 row-chunks on 4 queues (a single [83 x
    2 KB] store serializes ~95 ns/packet on one DMA engine: ~8 us).
"""

import numpy as np
from contextlib import ExitStack

import concourse.bass as bass
import concourse.bacc as bacc
import concourse.tile as tile
import concourse.mybir as mybir
from concourse import masks
from concourse.bass_utils import run_bass_kernel_spmd

F32 = mybir.dt.float32
BF16 = mybir.dt.bfloat16
AX = mybir.AxisListType
OP = mybir.AluOpType
ACT = mybir.ActivationFunctionType

N_CORES = 8
B, S, C, H, W = 4096, 15, 16, 6, 6
CK = C * H * W            # 576
SCK = S * CK              # 8640
NH, NL = 27, 28
O = 2 * NL + NH           # 83
BS = B // N_CORES         # 512 volumes per core
P = 128                   # partition tile
NT = BS // P              # 4 batch tiles per core
KC = [128, 128, 128, 128, 64]  # contraction chunking of 576
# Big chunks keep the completion-semaphore rotation short (only ~8 sems
# exist; DMA k's issue waits for DMA k-8's completion). Tile 3's tail
# slices go on the scalar/gpsimd queues: the last descriptors of a queue
# end up batched onto 1-2 DMA engines and trickle at ~26 GB/s, so they
# must drain in parallel with the main stream, not after it.


def _build_body(ctx, tc, o_d, x_d, s_d):
    nc = tc.nc

    const = ctx.enter_context(tc.tile_pool(name="const", bufs=1))
    ptr = ctx.enter_context(tc.tile_pool(name="ptr", bufs=4, space="PSUM"))
    pout = ctx.enter_context(tc.tile_pool(name="pout", bufs=2, space="PSUM"))
    xpool = ctx.enter_context(tc.tile_pool(name="xp", bufs=1))
    apool = ctx.enter_context(tc.tile_pool(name="ap", bufs=1))
    spool = ctx.enter_context(tc.tile_pool(name="sp", bufs=1))
    sspool = ctx.enter_context(tc.tile_pool(name="ssp", bufs=2))
    xtp = ctx.enter_context(tc.tile_pool(name="xtp", bufs=2))

    # ---- DMA schedule ---------------------------------------------------
    # sync queue: t0 chunk0 first, then the setup tensor (4 row-chunks; a
    # single [83 x 2.3 KB] DMA serializes on one engine for ~11 us), then
    # the remaining big chunks in tile order.
    LB = NT - 1
    xq_ = {}
    def xdma(eng, t, s0, ns, rsplit=1):
        # rsplit>1: issue as row-groups. A single dma_start's descriptors
        # get batched onto only ~2 DMA engines; chunks that drain at the
        # END of the queue (no other work in flight) would trickle at
        # ~50 GB/s. Row-split sub-DMAs fan out across engines.
        xq = xpool.tile([P, ns * CK], F32, tag=f"xq{t}_{s0}")
        rp = P // rsplit
        for r0 in range(0, P, rp):
            eng.dma_start(xq[r0:r0 + rp, :],
                          x_d[t * P + r0:t * P + r0 + rp,
                              s0 * CK:(s0 + ns) * CK])
        xq_[(t, s0)] = xq

    for (s0, ns) in ((0, 4), (4, 4), (8, 4), (12, 3)):
        xdma(nc.sync, 0, s0, ns)
    xdma(nc.sync, 1, 0, 4)
    xdma(nc.sync, 1, 4, 4)
    setup = const.tile([O, 16 + CK + 1], F32)
    for r0 in range(0, O, 21):
        nr = min(21, O - r0)
        nc.sync.dma_start(setup[r0:r0 + nr, :], s_d[r0:r0 + nr, :])
    attn = setup[:, 0:16]
    wsb = setup[:, 16:16 + CK]
    bias = setup[:, 16 + CK:16 + CK + 1]

    xdma(nc.scalar, LB, 12, 2)
    xdma(nc.gpsimd, LB, 14, 1)
    # side queues run ~50 GB/s (few engine batches) in parallel with the
    # main stream; park tile 2's late slices there to shorten the sync
    # stream (~2 MB = ~4.7 us at roofline).
    xdma(nc.scalar, 2, 12, 3)
    xdma(nc.gpsimd, 2, 8, 4)
    xdma(nc.sync, 1, 8, 4)
    xdma(nc.sync, 1, 12, 3)
    xdma(nc.sync, 2, 0, 4)
    xdma(nc.sync, 2, 4, 4)
    # tile 3 tail tapers: the queue's final DMAs lose engine parallelism
    # (~2 engines per 128-descriptor DMA), so the last slices get smaller
    # chunks and the final one is row-split 4 ways.
    xdma(nc.sync, LB, 0, 4)
    xdma(nc.sync, LB, 4, 4)
    xdma(nc.sync, LB, 8, 2)
    xdma(nc.sync, LB, 10, 1)
    xdma(nc.sync, LB, 11, 1, rsplit=4)

    # ---- constants / setup ---------------------------------------------
    ident = const.tile([128, 128], BF16)
    masks.make_identity(nc, ident[:])

    # softmax over the 16 channels, folded with the 1/S slice-average
    negmax = const.tile([O, 1], F32)
    nc.vector.tensor_reduce(negmax[:], attn, axis=AX.X, op=OP.max, negate=True)
    att_e = const.tile([O, 16], F32)
    den = const.tile([O, 1], F32)
    nc.scalar.activation(att_e[:], attn, ACT.Exp, bias=negmax[:, :], scale=1.0,
                         accum_out=den[:])
    den_s = const.tile([O, 1], F32)
    nc.scalar.mul(den_s[:], den[:], float(S))
    rden = const.tile([O, 1], F32)
    nc.vector.reciprocal(rden[:], den_s[:])
    attn_n = const.tile([O, 16], F32)
    nc.vector.tensor_scalar_mul(attn_n[:], att_e[:], rden[:, :])

    # Weff[o, c, k] = attn_n[o, c] * Wfc[o, c, k]   (k = 36 spatial), bf16
    weff = const.tile([O, CK], BF16)
    w_v = wsb.rearrange("p (c k) -> p c k", c=C)
    a_v = attn_n[:].rearrange("p (c k) -> p c k", k=1)
    o_v = weff[:].rearrange("p (c k) -> p c k", c=C)
    w_bc, a_bc = bass.broadcast_tensor_aps(w_v, a_v)
    nc.vector.tensor_tensor(o_v, w_bc, a_bc, op=OP.mult)

    # Weff^T chunks (bf16): wT[:, k*O:(k+1)*O] = Weff[:, chunk].T  ([kw, 83])
    wT = const.tile([128, 5 * O], BF16)
    for k, kw in enumerate(KC):
        c0 = 128 * k
        pt = ptr.tile([128, 128], BF16, tag="pt")
        nc.tensor.transpose(pt[0:kw, 0:O], weff[:, c0:c0 + kw], ident[0:O, 0:O])
        nc.scalar.copy(wT[0:kw, k * O:(k + 1) * O], pt[0:kw, 0:O])

    outsb = const.tile([O, BS], F32)

    # ---- main loop over batch tiles ------------------------------------
    # Slice-sum tree mostly on DVE, bf16 after the first level. GPSIMD
    # only runs SMALL ops (s12..s14 for tiles 0-2): its big fused ops
    # starve DVE (concurrent DVE [576] adds measured 8x slower). The last
    # tile's s12..s14 work runs on DVE but is emitted right after tile
    # 0's tree (its data arrives early on the side queues), keeping the
    # post-stream critical chain short.
    def views(t):
        if t == LB:
            q2a = xq_[(t, 8)][:].rearrange("p (s k) -> p s k", s=2)
            s8_11 = [q2a[:, 0, :], q2a[:, 1, :], xq_[(t, 10)][:], xq_[(t, 11)][:]]
            q3 = xq_[(t, 12)][:].rearrange("p (s k) -> p s k", s=2)
            s12_14 = [q3[:, 0, :], q3[:, 1, :], xq_[(t, 14)][:]]
        else:
            q2 = xq_[(t, 8)][:].rearrange("p (s k) -> p s k", s=4)
            s8_11 = [q2[:, j, :] for j in range(4)]
            q3 = xq_[(t, 12)][:].rearrange("p (s k) -> p s k", s=3)
            s12_14 = [q3[:, j, :] for j in range(3)]
        return s8_11, s12_14

    def tail_pair(t, eng, tagsuf=""):
        # a6 = s12+s13 ; b3 = a6+s14
        _, s12_14 = views(t)
        a6 = spool.tile([P, CK], BF16, tag="a6" + tagsuf)
        eng.tensor_add(a6[:], s12_14[0], s12_14[1])
        b3t = spool.tile([P, CK], BF16, tag="b3" + tagsuf)
        eng.tensor_add(b3t[:], a6[:], s12_14[2])
        return b3t

    def tree(t, b3t):
        q0 = xq_[(t, 0)]
        q1 = xq_[(t, 4)]
        s8_11, _ = views(t)
        a01 = apool.tile([P, 2 * CK], BF16, tag="a01")
        v0 = q0[:].rearrange("p (a b k) -> p a b k", a=2, b=2)
        nc.vector.tensor_tensor(a01[:].rearrange("p (a k) -> p a k", a=2),
                                v0[:, :, 0, :], v0[:, :, 1, :], op=OP.add)
        a23 = apool.tile([P, 2 * CK], BF16, tag="a23")
        v1 = q1[:].rearrange("p (a b k) -> p a b k", a=2, b=2)
        nc.vector.tensor_tensor(a23[:].rearrange("p (a k) -> p a k", a=2),
                                v1[:, :, 0, :], v1[:, :, 1, :], op=OP.add)
        b0t = spool.tile([P, CK], BF16, tag="b0")
        nc.vector.tensor_add(b0t[:], a01[:, 0:CK], a01[:, CK:2 * CK])
        b1t = spool.tile([P, CK], BF16, tag="b1")
        nc.vector.tensor_add(b1t[:], a23[:, 0:CK], a23[:, CK:2 * CK])
        c0t = spool.tile([P, CK], BF16, tag="c0")
        nc.vector.tensor_add(c0t[:], b0t[:], b1t[:])
        a4 = spool.tile([P, CK], BF16, tag="a4")
        nc.vector.tensor_add(a4[:], s8_11[0], s8_11[1])
        a5 = spool.tile([P, CK], BF16, tag="a5")
        nc.vector.tensor_add(a5[:], s8_11[2], s8_11[3])
        b2t = spool.tile([P, CK], BF16, tag="b2")
        nc.vector.tensor_add(b2t[:], a4[:], a5[:])
        c1t = spool.tile([P, CK], BF16, tag="c1")
        nc.vector.tensor_add(c1t[:], b2t[:], b3t[:])
        ss = sspool.tile([P, CK], BF16, tag="ss")
        nc.vector.tensor_add(ss[:], c0t[:], c1t[:])
        return ss

    b3_last = None
    ss_of = {}
    for t in range(NT):
        b0 = t * P
        if t == LB:
            b3t = b3_last
        else:
            b3t = tail_pair(t, nc.gpsimd)
        ss = tree(t, b3t)
        if t == 0:
            b3_last = tail_pair(LB, nc.vector, tagsuf="L")

        # PE transpose the slice-sum: xT[:, k*P:(k+1)*P] = ss[:, chunk].T
        xT = xtp.tile([128, 5 * P], BF16)
        for k, kw in enumerate(KC):
            c0 = 128 * k
            pt = ptr.tile([128, 128], BF16, tag="pt")
            nc.tensor.transpose(pt[0:kw, :], ss[:, c0:c0 + kw], ident[:, :])
            if k % 2 == 0:
                nc.scalar.copy(xT[0:kw, k * P:(k + 1) * P], pt[0:kw, :])
            else:
                nc.vector.tensor_copy(xT[0:kw, k * P:(k + 1) * P], pt[0:kw, :])

        po = pout.tile([O, P], F32)
        for k, kw in enumerate(KC):
            nc.tensor.matmul(po[:], wT[0:kw, k * O:(k + 1) * O],
                             xT[0:kw, k * P:(k + 1) * P],
                             start=(k == 0), stop=(k == len(KC) - 1))

        nc.scalar.add(outsb[:, b0:b0 + P], po[:], bias)

    # ---- output store: 3 row-chunks on the 3 DMA-capable queues --------
    RS = [(0, 28), (28, 28), (56, 27)]
    engines = [nc.sync, nc.scalar, nc.gpsimd]
    for (r0, nr), eng in zip(RS, engines):
        eng.dma_start(o_d[r0:r0 + nr, :], outsb[r0:r0 + nr, :])


def build_program(repeat: int = 1):
    nc = bacc.Bacc("TRN2", target_bir_lowering=False, debug=False,
                   num_devices=N_CORES)
    x_d = nc.dram_tensor("x", [BS, SCK], F32, kind="ExternalInput").ap()
    s_d = nc.dram_tensor("setup", [O, 16 + CK + 1], F32,
                         kind="ExternalInput").ap()
    o_d = nc.dram_tensor("out", [O, BS], F32, kind="ExternalOutput").ap()

    with tile.TileContext(nc) as tc:
        if repeat == 1:
            with ExitStack() as ctx:
                _build_body(ctx, tc, o_d, x_d, s_d)
        else:
            def body(_iv):
                with ExitStack() as ctx:
                    _build_body(ctx, tc, o_d, x_d, s_d)
            tc.For_i_unrolled(0, repeat, 1, body, max_unroll=1)
    nc.compile()
    return nc


_NC_CACHE = {}


def _get_program(repeat: int = 1):
    if repeat not in _NC_CACHE:
        _NC_CACHE[repeat] = build_program(repeat)
    return _NC_CACHE[repeat]


class _Runner:
    """Cached jitted shard_map runner (mirrors bass2jax.run_bass_via_pjrt's
    multi-core path, but built once and fed full arrays without the per-core
    split + re-concat host copies)."""

    def __init__(self, nc):
        import jax
        from jax.sharding import Mesh, PartitionSpec, NamedSharding
        from jax.experimental.shard_map import shard_map
        from concourse import bass2jax
        from concourse.bass2jax import _bass_exec_p, install_neuronx_cc_hook

        install_neuronx_cc_hook()
        self.jax = jax
        pname = nc.partition_id_tensor.name if nc.partition_id_tensor else None
        in_names, out_names, out_avals, zeros = [], [], [], []
        for alloc in nc.m.functions[0].allocations:
            if not isinstance(alloc, mybir.MemoryLocationSet):
                continue
            name = alloc.memorylocations[0].name
            if alloc.kind == "ExternalInput":
                if name != pname:
                    in_names.append(name)
            elif alloc.kind == "ExternalOutput":
                shape = tuple(alloc.tensor_shape)
                dtype = mybir.dt.np(alloc.dtype)
                out_names.append(name)
                out_avals.append(jax.core.ShapedArray(shape, dtype))
                zeros.append(np.zeros((N_CORES * shape[0], *shape[1:]), dtype))
        self.in_names, self.out_names, self.zeros = in_names, out_names, zeros
        all_in = list(in_names) + list(out_names)
        if pname is not None:
            all_in.append(pname)

        def _body(*args):
            operands = list(args)
            if pname is not None:
                operands.append(bass2jax.partition_id_tensor())
            return tuple(_bass_exec_p.bind(
                *operands, out_avals=tuple(out_avals), in_names=tuple(all_in),
                out_names=tuple(out_names), lowering_input_output_aliases=(),
                sim_require_finite=True, sim_require_nnan=True, nc=nc))

        devices = jax.devices()[:N_CORES]
        mesh = Mesh(np.asarray(devices), ("core",))
        n_p, n_o = len(in_names), len(out_names)
        self.sharded = jax.jit(
            shard_map(_body, mesh=mesh,
                      in_specs=(PartitionSpec("core"),) * (n_p + n_o),
                      out_specs=(PartitionSpec("core"),) * n_o,
                      check_rep=False),
            donate_argnums=tuple(range(n_p, n_p + n_o)), keep_unused=True)
        self.sharding = NamedSharding(mesh, PartitionSpec("core"))

    def __call__(self, full_ins: dict):
        outs = self.sharded(*[full_ins[n] for n in self.in_names],
                            *[z.copy() for z in self.zeros])
        return {n: np.asarray(outs[i]) for i, n in enumerate(self.out_names)}


_RUNNER = None


def _pack_setup(inputs):
    """[83, 593]: cols 0:16 attn logits (heart;lung;lung), 16:592 fcw, 592 bias."""
    h = np.asarray(inputs["dzfeatweights_heart"], dtype=np.float32).reshape(NH, 16)
    l = np.asarray(inputs["dzfeatweights_lung"], dtype=np.float32).reshape(NL, 16)
    w = np.asarray(inputs["fclayers_weights"], dtype=np.float32).reshape(O, CK)
    b = np.asarray(inputs["fclayers_biases"], dtype=np.float32).reshape(O, 1)
    return np.concatenate([np.concatenate([h, l, l], axis=0), w, b],
                          axis=1).astype(np.float32)


def make_in_maps(inputs):
    x = np.asarray(inputs["x"], dtype=np.float32).reshape(B, SCK)
    s = _pack_setup(inputs)
    return [{"x": x[c * BS:(c + 1) * BS], "setup": s} for c in range(N_CORES)]


def assemble_output(results):
    outs = [results[c]["out"] for c in range(N_CORES)]    # each [83, 512]
    return np.ascontiguousarray(np.concatenate(outs, axis=1).T)  # [4096, 83]


def kernel(**inputs) -> np.ndarray:
    global _RUNNER
    if _RUNNER is None:
        _RUNNER = _Runner(_get_program(1))
    # Full (concatenated-over-cores) input arrays; x needs no copy at all.
    full = {
        "x": np.ascontiguousarray(
            np.asarray(inputs["x"], dtype=np.float32)).reshape(B, SCK),
        "setup": np.tile(_pack_setup(inputs), (N_CORES, 1)),
    }
    outs = _RUNNER(full)["out"]            # [8*83, 512]
    per_core = outs.reshape(N_CORES, O, BS)
    return np.ascontiguousarray(
        np.concatenate([per_core[c] for c in range(N_CORES)], axis=1).T)


# revision 15
# speedup vs baseline: 1.0451x; 1.0451x over previous
"""Trainium2 Bass kernel for nn_BodyAvgDiseaseFeatureAttn2.

Computation (reference):
    attn  = softmax over channels of [heart(27); lung(28); lung(28)] -> [83, 16]
    Weff[o,c,h,w] = attn[o,c] * Wfc[o,c,h,w]
    out[b,o] = mean_s( sum_{c,h,w} x[b,s,c,h,w] * Weff[o,c,h,w] ) + bias[o]

Kernel strategy (pure data parallel, 8 cores, batch-sharded), v2:
  - x per core is 17.7 MB; HBM stream at ~428 GB/s is the roofline
    (~41.5 us). All 4 batch tiles (138 KB/partition) fit in SBUF, so ALL
    x DMAs are issued up front (20 chunk DMAs, no buffer reuse -> no
    stalls), and compute overlaps the stream with large headroom.
  - slice-sum tree: fp32 pair-adds split DVE (chunks 0,1 + combines) /
    GPSIMD (chunk 2 + tail combines), fused via 3D APs (2 pairs/op).
  - final 576-dot on PE in BF16 (tree output downcast in its last add;
    Weff precomputed in bf16): transposes+matmuls ~2x faster, fp32 PSUM
    accumulation keeps error ~1e-3 << 2e-2 budget.
  - psum->sbuf copies and the bias add run on the Scalar engine.
  - output store split into 4 row-chunks on 4 queues (a single [83 x
    2 KB] store serializes ~95 ns/packet on one DMA engine: ~8 us).
"""

import numpy as np
from contextlib import ExitStack

import concourse.bass as bass
import concourse.bacc as bacc
import concourse.tile as tile
import concourse.mybir as mybir
from concourse import masks
from concourse.bass_utils import run_bass_kernel_spmd

F32 = mybir.dt.float32
BF16 = mybir.dt.bfloat16
AX = mybir.AxisListType
OP = mybir.AluOpType
ACT = mybir.ActivationFunctionType

N_CORES = 8
B, S, C, H, W = 4096, 15, 16, 6, 6
CK = C * H * W            # 576
SCK = S * CK              # 8640
NH, NL = 27, 28
O = 2 * NL + NH           # 83
BS = B // N_CORES         # 512 volumes per core
P = 128                   # partition tile
NT = BS // P              # 4 batch tiles per core
KC = [128, 128, 128, 128, 64]  # contraction chunking of 576
# Big chunks keep the completion-semaphore rotation short (only ~8 sems
# exist; DMA k's issue waits for DMA k-8's completion). Tile 3's tail
# slices go on the scalar/gpsimd queues: the last descriptors of a queue
# end up batched onto 1-2 DMA engines and trickle at ~26 GB/s, so they
# must drain in parallel with the main stream, not after it.


def _build_body(ctx, tc, o_d, x_d, s_d):
    nc = tc.nc

    const = ctx.enter_context(tc.tile_pool(name="const", bufs=1))
    ptr = ctx.enter_context(tc.tile_pool(name="ptr", bufs=4, space="PSUM"))
    pout = ctx.enter_context(tc.tile_pool(name="pout", bufs=2, space="PSUM"))
    xpool = ctx.enter_context(tc.tile_pool(name="xp", bufs=1))
    apool = ctx.enter_context(tc.tile_pool(name="ap", bufs=1))
    spool = ctx.enter_context(tc.tile_pool(name="sp", bufs=1))
    sspool = ctx.enter_context(tc.tile_pool(name="ssp", bufs=2))
    xtp = ctx.enter_context(tc.tile_pool(name="xtp", bufs=2))

    # ---- DMA schedule ---------------------------------------------------
    # sync queue: t0 chunk0 first, then the setup tensor (4 row-chunks; a
    # single [83 x 2.3 KB] DMA serializes on one engine for ~11 us), then
    # the remaining big chunks in tile order.
    LB = NT - 1
    xq_ = {}
    def xdma(eng, t, s0, ns, rsplit=1):
        # rsplit>1: issue as row-groups. A single dma_start's descriptors
        # get batched onto only ~2 DMA engines; chunks that drain at the
        # END of the queue (no other work in flight) would trickle at
        # ~50 GB/s. Row-split sub-DMAs fan out across engines.
        xq = xpool.tile([P, ns * CK], F32, tag=f"xq{t}_{s0}")
        rp = P // rsplit
        for r0 in range(0, P, rp):
            eng.dma_start(xq[r0:r0 + rp, :],
                          x_d[t * P + r0:t * P + r0 + rp,
                              s0 * CK:(s0 + ns) * CK])
        xq_[(t, s0)] = xq

    for (s0, ns) in ((0, 4), (4, 4), (8, 4), (12, 3)):
        xdma(nc.sync, 0, s0, ns)
    xdma(nc.sync, 1, 0, 4)
    xdma(nc.sync, 1, 4, 4)
    setup = const.tile([O, 16 + CK + 1], F32)
    for r0 in range(0, O, 21):
        nr = min(21, O - r0)
        nc.sync.dma_start(setup[r0:r0 + nr, :], s_d[r0:r0 + nr, :])
    attn = setup[:, 0:16]
    wsb = setup[:, 16:16 + CK]
    bias = setup[:, 16 + CK:16 + CK + 1]

    xdma(nc.scalar, LB, 12, 2)
    xdma(nc.gpsimd, LB, 14, 1)
    xdma(nc.sync, 1, 8, 4)
    xdma(nc.sync, 1, 12, 3)
    for (s0, ns) in ((0, 4), (4, 4), (8, 4), (12, 3)):
        xdma(nc.sync, 2, s0, ns)
    # tile 3 tail tapers: the queue's final DMAs lose engine parallelism
    # (~2 engines per 128-descriptor DMA), so the last slices get smaller
    # chunks and the final one is row-split 4 ways.
    xdma(nc.sync, LB, 0, 4)
    xdma(nc.sync, LB, 4, 4)
    xdma(nc.sync, LB, 8, 2)
    xdma(nc.sync, LB, 10, 1)
    xdma(nc.sync, LB, 11, 1, rsplit=4)

    # ---- constants / setup ---------------------------------------------
    ident = const.tile([128, 128], BF16)
    masks.make_identity(nc, ident[:])

    # softmax over the 16 channels, folded with the 1/S slice-average
    negmax = const.tile([O, 1], F32)
    nc.vector.tensor_reduce(negmax[:], attn, axis=AX.X, op=OP.max, negate=True)
    att_e = const.tile([O, 16], F32)
    den = const.tile([O, 1], F32)
    nc.scalar.activation(att_e[:], attn, ACT.Exp, bias=negmax[:, :], scale=1.0,
                         accum_out=den[:])
    den_s = const.tile([O, 1], F32)
    nc.scalar.mul(den_s[:], den[:], float(S))
    rden = const.tile([O, 1], F32)
    nc.vector.reciprocal(rden[:], den_s[:])
    attn_n = const.tile([O, 16], F32)
    nc.vector.tensor_scalar_mul(attn_n[:], att_e[:], rden[:, :])

    # Weff[o, c, k] = attn_n[o, c] * Wfc[o, c, k]   (k = 36 spatial), bf16
    weff = const.tile([O, CK], BF16)
    w_v = wsb.rearrange("p (c k) -> p c k", c=C)
    a_v = attn_n[:].rearrange("p (c k) -> p c k", k=1)
    o_v = weff[:].rearrange("p (c k) -> p c k", c=C)
    w_bc, a_bc = bass.broadcast_tensor_aps(w_v, a_v)
    nc.vector.tensor_tensor(o_v, w_bc, a_bc, op=OP.mult)

    # Weff^T chunks (bf16): wT[:, k*O:(k+1)*O] = Weff[:, chunk].T  ([kw, 83])
    wT = const.tile([128, 5 * O], BF16)
    for k, kw in enumerate(KC):
        c0 = 128 * k
        pt = ptr.tile([128, 128], BF16, tag="pt")
        nc.tensor.transpose(pt[0:kw, 0:O], weff[:, c0:c0 + kw], ident[0:O, 0:O])
        nc.scalar.copy(wT[0:kw, k * O:(k + 1) * O], pt[0:kw, 0:O])

    outsb = const.tile([O, BS], F32)

    # ---- main loop over batch tiles ------------------------------------
    # Slice-sum tree mostly on DVE, bf16 after the first level. GPSIMD
    # only runs SMALL ops (s12..s14 for tiles 0-2): its big fused ops
    # starve DVE (concurrent DVE [576] adds measured 8x slower). The last
    # tile's s12..s14 work runs on DVE but is emitted right after tile
    # 0's tree (its data arrives early on the side queues), keeping the
    # post-stream critical chain short.
    def views(t):
        if t == LB:
            q2a = xq_[(t, 8)][:].rearrange("p (s k) -> p s k", s=2)
            s8_11 = [q2a[:, 0, :], q2a[:, 1, :], xq_[(t, 10)][:], xq_[(t, 11)][:]]
            q3 = xq_[(t, 12)][:].rearrange("p (s k) -> p s k", s=2)
            s12_14 = [q3[:, 0, :], q3[:, 1, :], xq_[(t, 14)][:]]
        else:
            q2 = xq_[(t, 8)][:].rearrange("p (s k) -> p s k", s=4)
            s8_11 = [q2[:, j, :] for j in range(4)]
            q3 = xq_[(t, 12)][:].rearrange("p (s k) -> p s k", s=3)
            s12_14 = [q3[:, j, :] for j in range(3)]
        return s8_11, s12_14

    def tail_pair(t, eng, tagsuf=""):
        # a6 = s12+s13 ; b3 = a6+s14
        _, s12_14 = views(t)
        a6 = spool.tile([P, CK], BF16, tag="a6" + tagsuf)
        eng.tensor_add(a6[:], s12_14[0], s12_14[1])
        b3t = spool.tile([P, CK], BF16, tag="b3" + tagsuf)
        eng.tensor_add(b3t[:], a6[:], s12_14[2])
        return b3t

    def tree(t, b3t):
        q0 = xq_[(t, 0)]
        q1 = xq_[(t, 4)]
        s8_11, _ = views(t)
        a01 = apool.tile([P, 2 * CK], BF16, tag="a01")
        v0 = q0[:].rearrange("p (a b k) -> p a b k", a=2, b=2)
        nc.vector.tensor_tensor(a01[:].rearrange("p (a k) -> p a k", a=2),
                                v0[:, :, 0, :], v0[:, :, 1, :], op=OP.add)
        a23 = apool.tile([P, 2 * CK], BF16, tag="a23")
        v1 = q1[:].rearrange("p (a b k) -> p a b k", a=2, b=2)
        nc.vector.tensor_tensor(a23[:].rearrange("p (a k) -> p a k", a=2),
                                v1[:, :, 0, :], v1[:, :, 1, :], op=OP.add)
        b0t = spool.tile([P, CK], BF16, tag="b0")
        nc.vector.tensor_add(b0t[:], a01[:, 0:CK], a01[:, CK:2 * CK])
        b1t = spool.tile([P, CK], BF16, tag="b1")
        nc.vector.tensor_add(b1t[:], a23[:, 0:CK], a23[:, CK:2 * CK])
        c0t = spool.tile([P, CK], BF16, tag="c0")
        nc.vector.tensor_add(c0t[:], b0t[:], b1t[:])
        a4 = spool.tile([P, CK], BF16, tag="a4")
        nc.vector.tensor_add(a4[:], s8_11[0], s8_11[1])
        a5 = spool.tile([P, CK], BF16, tag="a5")
        nc.vector.tensor_add(a5[:], s8_11[2], s8_11[3])
        b2t = spool.tile([P, CK], BF16, tag="b2")
        nc.vector.tensor_add(b2t[:], a4[:], a5[:])
        c1t = spool.tile([P, CK], BF16, tag="c1")
        nc.vector.tensor_add(c1t[:], b2t[:], b3t[:])
        ss = sspool.tile([P, CK], BF16, tag="ss")
        nc.vector.tensor_add(ss[:], c0t[:], c1t[:])
        return ss

    b3_last = None
    ss_of = {}
    for t in range(NT):
        b0 = t * P
        if t == LB:
            b3t = b3_last
        else:
            b3t = tail_pair(t, nc.gpsimd)
        ss = tree(t, b3t)
        if t == 0:
            b3_last = tail_pair(LB, nc.vector, tagsuf="L")

        # PE transpose the slice-sum: xT[:, k*P:(k+1)*P] = ss[:, chunk].T
        xT = xtp.tile([128, 5 * P], BF16)
        for k, kw in enumerate(KC):
            c0 = 128 * k
            pt = ptr.tile([128, 128], BF16, tag="pt")
            nc.tensor.transpose(pt[0:kw, :], ss[:, c0:c0 + kw], ident[:, :])
            if k % 2 == 0:
                nc.scalar.copy(xT[0:kw, k * P:(k + 1) * P], pt[0:kw, :])
            else:
                nc.vector.tensor_copy(xT[0:kw, k * P:(k + 1) * P], pt[0:kw, :])

        po = pout.tile([O, P], F32)
        for k, kw in enumerate(KC):
            nc.tensor.matmul(po[:], wT[0:kw, k * O:(k + 1) * O],
                             xT[0:kw, k * P:(k + 1) * P],
                             start=(k == 0), stop=(k == len(KC) - 1))

        nc.vector.tensor_scalar_add(outsb[:, b0:b0 + P], po[:], bias)

    # ---- output store: 3 row-chunks on the 3 DMA-capable queues --------
    RS = [(0, 28), (28, 28), (56, 27)]
    engines = [nc.sync, nc.scalar, nc.gpsimd]
    for (r0, nr), eng in zip(RS, engines):
        eng.dma_start(o_d[r0:r0 + nr, :], outsb[r0:r0 + nr, :])


def build_program(repeat: int = 1):
    nc = bacc.Bacc("TRN2", target_bir_lowering=False, debug=False,
                   num_devices=N_CORES)
    x_d = nc.dram_tensor("x", [BS, SCK], F32, kind="ExternalInput").ap()
    s_d = nc.dram_tensor("setup", [O, 16 + CK + 1], F32,
                         kind="ExternalInput").ap()
    o_d = nc.dram_tensor("out", [O, BS], F32, kind="ExternalOutput").ap()

    with tile.TileContext(nc) as tc:
        if repeat == 1:
            with ExitStack() as ctx:
                _build_body(ctx, tc, o_d, x_d, s_d)
        else:
            def body(_iv):
                with ExitStack() as ctx:
                    _build_body(ctx, tc, o_d, x_d, s_d)
            tc.For_i_unrolled(0, repeat, 1, body, max_unroll=1)
    nc.compile()
    return nc


_NC_CACHE = {}


def _get_program(repeat: int = 1):
    if repeat not in _NC_CACHE:
        _NC_CACHE[repeat] = build_program(repeat)
    return _NC_CACHE[repeat]


class _Runner:
    """Cached jitted shard_map runner (mirrors bass2jax.run_bass_via_pjrt's
    multi-core path, but built once and fed full arrays without the per-core
    split + re-concat host copies)."""

    def __init__(self, nc):
        import jax
        from jax.sharding import Mesh, PartitionSpec, NamedSharding
        from jax.experimental.shard_map import shard_map
        from concourse import bass2jax
        from concourse.bass2jax import _bass_exec_p, install_neuronx_cc_hook

        install_neuronx_cc_hook()
        self.jax = jax
        pname = nc.partition_id_tensor.name if nc.partition_id_tensor else None
        in_names, out_names, out_avals, zeros = [], [], [], []
        for alloc in nc.m.functions[0].allocations:
            if not isinstance(alloc, mybir.MemoryLocationSet):
                continue
            name = alloc.memorylocations[0].name
            if alloc.kind == "ExternalInput":
                if name != pname:
                    in_names.append(name)
            elif alloc.kind == "ExternalOutput":
                shape = tuple(alloc.tensor_shape)
                dtype = mybir.dt.np(alloc.dtype)
                out_names.append(name)
                out_avals.append(jax.core.ShapedArray(shape, dtype))
                zeros.append(np.zeros((N_CORES * shape[0], *shape[1:]), dtype))
        self.in_names, self.out_names, self.zeros = in_names, out_names, zeros
        all_in = list(in_names) + list(out_names)
        if pname is not None:
            all_in.append(pname)

        def _body(*args):
            operands = list(args)
            if pname is not None:
                operands.append(bass2jax.partition_id_tensor())
            return tuple(_bass_exec_p.bind(
                *operands, out_avals=tuple(out_avals), in_names=tuple(all_in),
                out_names=tuple(out_names), lowering_input_output_aliases=(),
                sim_require_finite=True, sim_require_nnan=True, nc=nc))

        devices = jax.devices()[:N_CORES]
        mesh = Mesh(np.asarray(devices), ("core",))
        n_p, n_o = len(in_names), len(out_names)
        self.sharded = jax.jit(
            shard_map(_body, mesh=mesh,
                      in_specs=(PartitionSpec("core"),) * (n_p + n_o),
                      out_specs=(PartitionSpec("core"),) * n_o,
                      check_rep=False),
            donate_argnums=tuple(range(n_p, n_p + n_o)), keep_unused=True)
        self.sharding = NamedSharding(mesh, PartitionSpec("core"))

    def __call__(self, full_ins: dict):
        outs = self.sharded(*[full_ins[n] for n in self.in_names],
                            *[z.copy() for z in self.zeros])
        return {n: np.asarray(outs[i]) for i, n in enumerate(self.out_names)}


_RUNNER = None


def _pack_setup(inputs):
    """[83, 593]: cols 0:16 attn logits (heart;lung;lung), 16:592 fcw, 592 bias."""
    h = np.asarray(inputs["dzfeatweights_heart"], dtype=np.float32).reshape(NH, 16)
    l = np.asarray(inputs["dzfeatweights_lung"], dtype=np.float32).reshape(NL, 16)
    w = np.asarray(inputs["fclayers_weights"], dtype=np.float32).reshape(O, CK)
    b = np.asarray(inputs["fclayers_biases"], dtype=np.float32).reshape(O, 1)
    return np.concatenate([np.concatenate([h, l, l], axis=0), w, b],
                          axis=1).astype(np.float32)


def make_in_maps(inputs):
    x = np.asarray(inputs["x"], dtype=np.float32).reshape(B, SCK)
    s = _pack_setup(inputs)
    return [{"x": x[c * BS:(c + 1) * BS], "setup": s} for c in range(N_CORES)]


def assemble_output(results):
    outs = [results[c]["out"] for c in range(N_CORES)]    # each [83, 512]
    return np.ascontiguousarray(np.concatenate(outs, axis=1).T)  # [4096, 83]


def kernel(**inputs) -> np.ndarray:
    global _RUNNER
    if _RUNNER is None:
        _RUNNER = _Runner(_get_program(1))
    # Full (concatenated-over-cores) input arrays; x needs no copy at all.
    full = {
        "x": np.ascontiguousarray(
            np.asarray(inputs["x"], dtype=np.float32)).reshape(B, SCK),
        "setup": np.tile(_pack_setup(inputs), (N_CORES, 1)),
    }
    outs = _RUNNER(full)["out"]            # [8*83, 512]
    per_core = outs.reshape(N_CORES, O, BS)
    return np.ascontiguousarray(
        np.concatenate([per_core[c] for c in range(N_CORES)], axis=1).T)


# revision 16
# speedup vs baseline: 1.0932x; 1.0461x over previous
"""Trainium2 Bass kernel for nn_BodyAvgDiseaseFeatureAttn2.

Computation (reference):
    attn  = softmax over channels of [heart(27); lung(28); lung(28)] -> [83, 16]
    Weff[o,c,h,w] = attn[o,c] * Wfc[o,c,h,w]
    out[b,o] = mean_s( sum_{c,h,w} x[b,s,c,h,w] * Weff[o,c,h,w] ) + bias[o]

Kernel strategy (pure data parallel, 8 cores, batch-sharded), v2:
  - x per core is 17.7 MB; HBM stream at ~428 GB/s is the roofline
    (~41.5 us). All 4 batch tiles (138 KB/partition) fit in SBUF, so ALL
    x DMAs are issued up front (20 chunk DMAs, no buffer reuse -> no
    stalls), and compute overlaps the stream with large headroom.
  - slice-sum tree: fp32 pair-adds split DVE (chunks 0,1 + combines) /
    GPSIMD (chunk 2 + tail combines), fused via 3D APs (2 pairs/op).
  - final 576-dot on PE in BF16 (tree output downcast in its last add;
    Weff precomputed in bf16): transposes+matmuls ~2x faster, fp32 PSUM
    accumulation keeps error ~1e-3 << 2e-2 budget.
  - psum->sbuf copies and the bias add run on the Scalar engine.
  - output store split into 4 row-chunks on 4 queues (a single [83 x
    2 KB] store serializes ~95 ns/packet on one DMA engine: ~8 us).
"""

import numpy as np
from contextlib import ExitStack

import concourse.bass as bass
import concourse.bacc as bacc
import concourse.tile as tile
import concourse.mybir as mybir
from concourse import masks
from concourse.bass_utils import run_bass_kernel_spmd

F32 = mybir.dt.float32
BF16 = mybir.dt.bfloat16
AX = mybir.AxisListType
OP = mybir.AluOpType
ACT = mybir.ActivationFunctionType

N_CORES = 8
B, S, C, H, W = 4096, 15, 16, 6, 6
CK = C * H * W            # 576
SCK = S * CK              # 8640
NH, NL = 27, 28
O = 2 * NL + NH           # 83
BS = B // N_CORES         # 512 volumes per core
P = 128                   # partition tile
NT = BS // P              # 4 batch tiles per core
KC = [128, 128, 128, 128, 64]  # contraction chunking of 576
# Big chunks keep the completion-semaphore rotation short (only ~8 sems
# exist; DMA k's issue waits for DMA k-8's completion). Tile 3's tail
# slices go on the scalar/gpsimd queues: the last descriptors of a queue
# end up batched onto 1-2 DMA engines and trickle at ~26 GB/s, so they
# must drain in parallel with the main stream, not after it.


def _build_body(ctx, tc, o_d, x_d, s_d):
    nc = tc.nc

    const = ctx.enter_context(tc.tile_pool(name="const", bufs=1))
    ptr = ctx.enter_context(tc.tile_pool(name="ptr", bufs=4, space="PSUM"))
    pout = ctx.enter_context(tc.tile_pool(name="pout", bufs=2, space="PSUM"))
    xpool = ctx.enter_context(tc.tile_pool(name="xp", bufs=1))
    spool = ctx.enter_context(tc.tile_pool(name="sp", bufs=2))
    apool = spool
    sspool = spool
    xtp = spool

    # ---- DMA schedule ---------------------------------------------------
    # sync queue: t0 chunk0 first, then the setup tensor (4 row-chunks; a
    # single [83 x 2.3 KB] DMA serializes on one engine for ~11 us), then
    # the remaining big chunks in tile order.
    LB = NT - 1
    xq_ = {}
    def xdma(eng, t, s0, ns, rsplit=1):
        # rsplit>1: issue as row-groups. A single dma_start's descriptors
        # get batched onto only ~2 DMA engines; chunks that drain at the
        # END of the queue (no other work in flight) would trickle at
        # ~50 GB/s. Row-split sub-DMAs fan out across engines.
        xq = xpool.tile([P, ns * CK], F32, tag=f"xq{t}_{s0}")
        rp = P // rsplit
        for r0 in range(0, P, rp):
            eng.dma_start(xq[r0:r0 + rp, :],
                          x_d[t * P + r0:t * P + r0 + rp,
                              s0 * CK:(s0 + ns) * CK])
        xq_[(t, s0)] = xq

    for (s0, ns) in ((0, 4), (4, 4), (8, 4), (12, 3)):
        xdma(nc.sync, 0, s0, ns)
    xdma(nc.sync, 1, 0, 4)
    xdma(nc.sync, 1, 4, 4)
    setup = const.tile([O, 16 + CK + 1], F32)
    for r0 in range(0, O, 21):
        nr = min(21, O - r0)
        nc.sync.dma_start(setup[r0:r0 + nr, :], s_d[r0:r0 + nr, :])
    attn = setup[:, 0:16]
    wsb = setup[:, 16:16 + CK]
    bias = setup[:, 16 + CK:16 + CK + 1]

    xdma(nc.scalar, LB, 12, 2)
    xdma(nc.gpsimd, LB, 14, 1)
    xdma(nc.sync, 1, 8, 4)
    xdma(nc.sync, 1, 12, 3)
    for (s0, ns) in ((0, 4), (4, 4), (8, 4), (12, 3)):
        xdma(nc.sync, 2, s0, ns)
    # tile 3 tail tapers: the queue's final DMAs lose engine parallelism
    # (~2 engines per 128-descriptor DMA), so the last slices get smaller
    # chunks and the final one is row-split 4 ways.
    xdma(nc.sync, LB, 0, 4)
    xdma(nc.sync, LB, 4, 4)
    xdma(nc.sync, LB, 8, 2)
    xdma(nc.sync, LB, 10, 1)
    xdma(nc.sync, LB, 11, 1, rsplit=4)

    # ---- constants / setup ---------------------------------------------
    ident = const.tile([128, 128], BF16)
    masks.make_identity(nc, ident[:])

    # softmax over the 16 channels, folded with the 1/S slice-average
    negmax = const.tile([O, 1], F32)
    nc.vector.tensor_reduce(negmax[:], attn, axis=AX.X, op=OP.max, negate=True)
    att_e = const.tile([O, 16], F32)
    den = const.tile([O, 1], F32)
    nc.scalar.activation(att_e[:], attn, ACT.Exp, bias=negmax[:, :], scale=1.0,
                         accum_out=den[:])
    den_s = const.tile([O, 1], F32)
    nc.scalar.mul(den_s[:], den[:], float(S))
    rden = const.tile([O, 1], F32)
    nc.vector.reciprocal(rden[:], den_s[:])
    attn_n = const.tile([O, 16], F32)
    nc.vector.tensor_scalar_mul(attn_n[:], att_e[:], rden[:, :])

    # Weff[o, c, k] = attn_n[o, c] * Wfc[o, c, k]   (k = 36 spatial), bf16
    weff = const.tile([O, CK], BF16)
    w_v = wsb.rearrange("p (c k) -> p c k", c=C)
    a_v = attn_n[:].rearrange("p (c k) -> p c k", k=1)
    o_v = weff[:].rearrange("p (c k) -> p c k", c=C)
    w_bc, a_bc = bass.broadcast_tensor_aps(w_v, a_v)
    nc.vector.tensor_tensor(o_v, w_bc, a_bc, op=OP.mult)

    # Weff^T chunks (bf16): wT[:, k*O:(k+1)*O] = Weff[:, chunk].T  ([kw, 83])
    wT = const.tile([128, 5 * O], BF16)
    for k, kw in enumerate(KC):
        c0 = 128 * k
        pt = ptr.tile([128, 128], BF16, tag="pt")
        nc.tensor.transpose(pt[0:kw, 0:O], weff[:, c0:c0 + kw], ident[0:O, 0:O])
        nc.scalar.copy(wT[0:kw, k * O:(k + 1) * O], pt[0:kw, 0:O])

    outsb = const.tile([O, BS], F32)

    # ---- main loop over batch tiles ------------------------------------
    # Slice-sum tree mostly on DVE, bf16 after the first level. GPSIMD
    # only runs SMALL ops (s12..s14 for tiles 0-2): its big fused ops
    # starve DVE (concurrent DVE [576] adds measured 8x slower). The last
    # tile's s12..s14 work runs on DVE but is emitted right after tile
    # 0's tree (its data arrives early on the side queues), keeping the
    # post-stream critical chain short.
    def views(t):
        if t == LB:
            q2a = xq_[(t, 8)][:].rearrange("p (s k) -> p s k", s=2)
            s8_11 = [q2a[:, 0, :], q2a[:, 1, :], xq_[(t, 10)][:], xq_[(t, 11)][:]]
            q3 = xq_[(t, 12)][:].rearrange("p (s k) -> p s k", s=2)
            s12_14 = [q3[:, 0, :], q3[:, 1, :], xq_[(t, 14)][:]]
        else:
            q2 = xq_[(t, 8)][:].rearrange("p (s k) -> p s k", s=4)
            s8_11 = [q2[:, j, :] for j in range(4)]
            q3 = xq_[(t, 12)][:].rearrange("p (s k) -> p s k", s=3)
            s12_14 = [q3[:, j, :] for j in range(3)]
        return s8_11, s12_14

    def tail_pair(t, eng, tagsuf=""):
        # a6 = s12+s13 ; b3 = a6+s14
        _, s12_14 = views(t)
        a6 = spool.tile([P, CK], BF16, tag="a6" + tagsuf)
        eng.tensor_add(a6[:], s12_14[0], s12_14[1])
        b3t = spool.tile([P, CK], BF16, tag="b3" + tagsuf)
        eng.tensor_add(b3t[:], a6[:], s12_14[2])
        return b3t

    def tree(t, b3t):
        q0 = xq_[(t, 0)]
        q1 = xq_[(t, 4)]
        s8_11, _ = views(t)
        a01 = apool.tile([P, 2 * CK], BF16, tag="a01")
        v0 = q0[:].rearrange("p (a b k) -> p a b k", a=2, b=2)
        nc.vector.tensor_tensor(a01[:].rearrange("p (a k) -> p a k", a=2),
                                v0[:, :, 0, :], v0[:, :, 1, :], op=OP.add)
        a23 = apool.tile([P, 2 * CK], BF16, tag="a23")
        v1 = q1[:].rearrange("p (a b k) -> p a b k", a=2, b=2)
        nc.vector.tensor_tensor(a23[:].rearrange("p (a k) -> p a k", a=2),
                                v1[:, :, 0, :], v1[:, :, 1, :], op=OP.add)
        b0t = spool.tile([P, CK], BF16, tag="b0")
        nc.vector.tensor_add(b0t[:], a01[:, 0:CK], a01[:, CK:2 * CK])
        b1t = spool.tile([P, CK], BF16, tag="b1")
        nc.vector.tensor_add(b1t[:], a23[:, 0:CK], a23[:, CK:2 * CK])
        c0t = spool.tile([P, CK], BF16, tag="c0")
        nc.vector.tensor_add(c0t[:], b0t[:], b1t[:])
        a4 = spool.tile([P, CK], BF16, tag="a4")
        nc.vector.tensor_add(a4[:], s8_11[0], s8_11[1])
        a5 = spool.tile([P, CK], BF16, tag="a5")
        nc.vector.tensor_add(a5[:], s8_11[2], s8_11[3])
        b2t = spool.tile([P, CK], BF16, tag="b2")
        nc.vector.tensor_add(b2t[:], a4[:], a5[:])
        c1t = spool.tile([P, CK], BF16, tag="c1")
        nc.vector.tensor_add(c1t[:], b2t[:], b3t[:])
        ss = sspool.tile([P, CK], BF16, tag="ss")
        nc.vector.tensor_add(ss[:], c0t[:], c1t[:])
        return ss

    b3_last = None
    ss_of = {}
    for t in range(NT):
        b0 = t * P
        if t == LB:
            b3t = b3_last
        else:
            b3t = tail_pair(t, nc.gpsimd)
        ss = tree(t, b3t)
        if t == 0:
            b3_last = tail_pair(LB, nc.vector, tagsuf="L")

        # PE transpose the slice-sum: xT[:, k*P:(k+1)*P] = ss[:, chunk].T
        xT = xtp.tile([128, 5 * P], BF16)
        for k, kw in enumerate(KC):
            c0 = 128 * k
            pt = ptr.tile([128, 128], BF16, tag="pt")
            nc.tensor.transpose(pt[0:kw, :], ss[:, c0:c0 + kw], ident[:, :])
            if k % 2 == 0:
                nc.scalar.copy(xT[0:kw, k * P:(k + 1) * P], pt[0:kw, :])
            else:
                nc.vector.tensor_copy(xT[0:kw, k * P:(k + 1) * P], pt[0:kw, :])

        po = pout.tile([O, P], F32)
        for k, kw in enumerate(KC):
            nc.tensor.matmul(po[:], wT[0:kw, k * O:(k + 1) * O],
                             xT[0:kw, k * P:(k + 1) * P],
                             start=(k == 0), stop=(k == len(KC) - 1))

        nc.vector.tensor_scalar_add(outsb[:, b0:b0 + P], po[:], bias)

    # ---- output store: 3 row-chunks on the 3 DMA-capable queues --------
    RS = [(0, 28), (28, 28), (56, 27)]
    engines = [nc.sync, nc.scalar, nc.gpsimd]
    for (r0, nr), eng in zip(RS, engines):
        eng.dma_start(o_d[r0:r0 + nr, :], outsb[r0:r0 + nr, :])


def build_program(repeat: int = 1):
    nc = bacc.Bacc("TRN2", target_bir_lowering=False, debug=False,
                   num_devices=N_CORES)
    x_d = nc.dram_tensor("x", [BS, SCK], F32, kind="ExternalInput").ap()
    s_d = nc.dram_tensor("setup", [O, 16 + CK + 1], F32,
                         kind="ExternalInput").ap()
    o_d = nc.dram_tensor("out", [O, BS], F32, kind="ExternalOutput").ap()

    with tile.TileContext(nc) as tc:
        if repeat == 1:
            with ExitStack() as ctx:
                _build_body(ctx, tc, o_d, x_d, s_d)
        else:
            def body(_iv):
                with ExitStack() as ctx:
                    _build_body(ctx, tc, o_d, x_d, s_d)
            tc.For_i_unrolled(0, repeat, 1, body, max_unroll=1)
    nc.compile()
    return nc


_NC_CACHE = {}


def _get_program(repeat: int = 1):
    if repeat not in _NC_CACHE:
        _NC_CACHE[repeat] = build_program(repeat)
    return _NC_CACHE[repeat]


class _Runner:
    """Cached jitted shard_map runner (mirrors bass2jax.run_bass_via_pjrt's
    multi-core path, but built once and fed full arrays without the per-core
    split + re-concat host copies)."""

    def __init__(self, nc):
        import jax
        from jax.sharding import Mesh, PartitionSpec, NamedSharding
        from jax.experimental.shard_map import shard_map
        from concourse import bass2jax
        from concourse.bass2jax import _bass_exec_p, install_neuronx_cc_hook

        install_neuronx_cc_hook()
        self.jax = jax
        pname = nc.partition_id_tensor.name if nc.partition_id_tensor else None
        in_names, out_names, out_avals, zeros = [], [], [], []
        for alloc in nc.m.functions[0].allocations:
            if not isinstance(alloc, mybir.MemoryLocationSet):
                continue
            name = alloc.memorylocations[0].name
            if alloc.kind == "ExternalInput":
                if name != pname:
                    in_names.append(name)
            elif alloc.kind == "ExternalOutput":
                shape = tuple(alloc.tensor_shape)
                dtype = mybir.dt.np(alloc.dtype)
                out_names.append(name)
                out_avals.append(jax.core.ShapedArray(shape, dtype))
                zeros.append(np.zeros((N_CORES * shape[0], *shape[1:]), dtype))
        self.in_names, self.out_names, self.zeros = in_names, out_names, zeros
        all_in = list(in_names) + list(out_names)
        if pname is not None:
            all_in.append(pname)

        def _body(*args):
            operands = list(args)
            if pname is not None:
                operands.append(bass2jax.partition_id_tensor())
            return tuple(_bass_exec_p.bind(
                *operands, out_avals=tuple(out_avals), in_names=tuple(all_in),
                out_names=tuple(out_names), lowering_input_output_aliases=(),
                sim_require_finite=True, sim_require_nnan=True, nc=nc))

        devices = jax.devices()[:N_CORES]
        mesh = Mesh(np.asarray(devices), ("core",))
        n_p, n_o = len(in_names), len(out_names)
        self.sharded = jax.jit(
            shard_map(_body, mesh=mesh,
                      in_specs=(PartitionSpec("core"),) * (n_p + n_o),
                      out_specs=(PartitionSpec("core"),) * n_o,
                      check_rep=False),
            donate_argnums=tuple(range(n_p, n_p + n_o)), keep_unused=True)
        self.sharding = NamedSharding(mesh, PartitionSpec("core"))

    def __call__(self, full_ins: dict):
        outs = self.sharded(*[full_ins[n] for n in self.in_names],
                            *[z.copy() for z in self.zeros])
        return {n: np.asarray(outs[i]) for i, n in enumerate(self.out_names)}


_RUNNER = None


def _pack_setup(inputs):
    """[83, 593]: cols 0:16 attn logits (heart;lung;lung), 16:592 fcw, 592 bias."""
    h = np.asarray(inputs["dzfeatweights_heart"], dtype=np.float32).reshape(NH, 16)
    l = np.asarray(inputs["dzfeatweights_lung"], dtype=np.float32).reshape(NL, 16)
    w = np.asarray(inputs["fclayers_weights"], dtype=np.float32).reshape(O, CK)
    b = np.asarray(inputs["fclayers_biases"], dtype=np.float32).reshape(O, 1)
    return np.concatenate([np.concatenate([h, l, l], axis=0), w, b],
                          axis=1).astype(np.float32)


def make_in_maps(inputs):
    x = np.asarray(inputs["x"], dtype=np.float32).reshape(B, SCK)
    s = _pack_setup(inputs)
    return [{"x": x[c * BS:(c + 1) * BS], "setup": s} for c in range(N_CORES)]


def assemble_output(results):
    outs = [results[c]["out"] for c in range(N_CORES)]    # each [83, 512]
    return np.ascontiguousarray(np.concatenate(outs, axis=1).T)  # [4096, 83]


def kernel(**inputs) -> np.ndarray:
    global _RUNNER
    if _RUNNER is None:
        _RUNNER = _Runner(_get_program(1))
    # Full (concatenated-over-cores) input arrays; x needs no copy at all.
    full = {
        "x": np.ascontiguousarray(
            np.asarray(inputs["x"], dtype=np.float32)).reshape(B, SCK),
        "setup": np.tile(_pack_setup(inputs), (N_CORES, 1)),
    }
    outs = _RUNNER(full)["out"]            # [8*83, 512]
    per_core = outs.reshape(N_CORES, O, BS)
    return np.ascontiguousarray(
        np.concatenate([per_core[c] for c in range(N_CORES)], axis=1).T)
